# revision 7
# baseline (speedup 1.0000x reference)
"""TRN2 Bass kernel for CrossAttentionBlock.

Reference computation (per batch b):
  q = (wq @ xf)          # [Co, N] -> used transposed
  k = wk @ yf            # [Co, N]
  v = wv @ yf            # [Co, N]
  energy[i, j] = sum_o q[o, i] * k[o, j]
  att = softmax_j(energy)
  out[c, i] = gamma * sum_j v[c, j] * att[i, j] + x[c, i]

Sharding: 8 cores = 4 batches x 2 query-halves. Each core handles the
2048 query rows of one half of one batch; the full [Co, N] k/v for that
batch are computed on-core (cheap projections, duplicated per pair).

On-core dataflow (all matmuls in float32r: 1 cycle/row at free-dim>=256,
~2e-4 relative error vs 2.7e-3 for bf16):
  - qT [o, i] and k [o, j]: contraction dim o lives on partitions.
  - energy computed transposed, eT [j_tile=128, i_blk] per j-tile, so the
    softmax exp can stream PSUM->SBUF through the ACT engine with a
    global-max subtraction (M=60; softmax is shift-invariant, rows cannot
    underflow to zero for this energy scale).
  - attention-weighted V accumulates in natural [c, i] layout across the
    j-loop: lhsT = vT[j, c_chunk], rhs = pT[j, i_blk].
  - row sums: DVE accumulates pT tiles elementwise; a ones-vector matmul
    reduces over partitions; reciprocal + a rank-1 ones matmul broadcasts
    1/s back across partitions; finalize fuses *1/s and +x on DVE.
  - gamma is folded into wv on the host.
"""

import numpy as np

B = 4
C = 256
N = 4096          # H * W
NQ = N // 2       # query rows per core
I_BLK = 512
N_IB = NQ // I_BLK   # 4 i-blocks
N_JT = N // 128      # 32 j-tiles
NEG_M = -60.0        # global softmax shift

_CACHE = {}


def _build():
    import concourse.tile as tile
    from concourse import bacc, mybir

    f32 = mybir.dt.float32
    f32r = mybir.dt.float32r
    Exp = mybir.ActivationFunctionType.Exp
    Mult = mybir.AluOpType.mult

    nc = bacc.Bacc("TRN2", target_bir_lowering=False, debug=False)

    xf_d = nc.dram_tensor("xf", [C, NQ], f32r, kind="ExternalInput")
    yf_d = nc.dram_tensor("yf", [C, N], f32r, kind="ExternalInput")
    wqT_d = nc.dram_tensor("wqT", [C, C], f32r, kind="ExternalInput")
    wkT_d = nc.dram_tensor("wkT", [C, C], f32r, kind="ExternalInput")
    wvT_d = nc.dram_tensor("wvT", [C, C], f32r, kind="ExternalInput")
    out_d = nc.dram_tensor("out", [C, NQ], f32, kind="ExternalOutput")

    with tile.TileContext(nc) as tc:
        with (
            tc.tile_pool(name="persist", bufs=1) as persist,
            tc.tile_pool(name="ptile", bufs=4) as ptile,
            tc.tile_pool(name="sacc_pool", bufs=2) as sacc_pool,
            tc.tile_pool(name="fin", bufs=4) as fin,
            tc.tile_pool(name="rrow", bufs=2) as rrow_pool,
            tc.tile_pool(name="mm512", bufs=2, space="PSUM") as mm512,
            tc.tile_pool(name="outps", bufs=4, space="PSUM") as outps,
            tc.tile_pool(name="sps", bufs=2, space="PSUM") as sps,
        ):
            # ---- load inputs ----
            xf = [persist.tile([128, NQ], f32r, tag=f"xf{cc}", name=f"xf{cc}") for cc in range(2)]
            yf = [persist.tile([128, N], f32r, tag=f"yf{cc}", name=f"yf{cc}") for cc in range(2)]
            wqT = [persist.tile([128, C], f32r, tag=f"wq{cc}", name=f"wq{cc}") for cc in range(2)]
            wkT = [persist.tile([128, C], f32r, tag=f"wk{cc}", name=f"wk{cc}") for cc in range(2)]
            wvT = [persist.tile([128, C], f32r, tag=f"wv{cc}", name=f"wv{cc}") for cc in range(2)]
            for cc in range(2):
                rows = slice(cc * 128, (cc + 1) * 128)
                nc.sync.dma_start(out=xf[cc][:], in_=xf_d[rows, :])
                nc.sync.dma_start(out=yf[cc][:], in_=yf_d[rows, :])
                nc.sync.dma_start(out=wqT[cc][:], in_=wqT_d[rows, :])
                nc.sync.dma_start(out=wkT[cc][:], in_=wkT_d[rows, :])
                nc.sync.dma_start(out=wvT[cc][:], in_=wvT_d[rows, :])

            ones_col = persist.tile([128, 1], f32, tag="ones_col", name="ones_col")
            nc.vector.memset(ones_col[:], 1.0)
            ones_row = persist.tile([1, 128], f32, tag="ones_row", name="ones_row")
            nc.vector.memset(ones_row[:], 1.0)
            neg_m = persist.tile([128, 1], f32, tag="neg_m", name="neg_m")
            nc.vector.memset(neg_m[:], NEG_M)

            # ---- projections ----
            qT = [persist.tile([128, NQ], f32r, tag=f"qT{oc}", name=f"qT{oc}") for oc in range(2)]
            k_sb = [persist.tile([128, N], f32r, tag=f"k{oc}", name=f"k{oc}") for oc in range(2)]
            vT = [persist.tile([128, C], f32r, tag=f"vT{nt}", name=f"vT{nt}") for nt in range(N_JT)]

            for oc in range(2):
                ocs = slice(oc * 128, (oc + 1) * 128)
                for it in range(N_IB):
                    s = slice(it * I_BLK, (it + 1) * I_BLK)
                    ps = mm512.tile([128, I_BLK], f32, tag="mm512", name="mm512")
                    nc.tensor.matmul(ps[:], wqT[0][:, ocs], xf[0][:, s],
                                     start=True, stop=False)
                    nc.tensor.matmul(ps[:], wqT[1][:, ocs], xf[1][:, s],
                                     start=False, stop=True)
                    nc.scalar.copy(qT[oc][:, s], ps[:])
                for jt8 in range(N // 512):
                    s = slice(jt8 * 512, (jt8 + 1) * 512)
                    ps = mm512.tile([128, 512], f32, tag="mm512", name="mm512")
                    nc.tensor.matmul(ps[:], wkT[0][:, ocs], yf[0][:, s],
                                     start=True, stop=False)
                    nc.tensor.matmul(ps[:], wkT[1][:, ocs], yf[1][:, s],
                                     start=False, stop=True)
                    nc.vector.tensor_copy(k_sb[oc][:, s], ps[:])
            for nt in range(N_JT):
                s = slice(nt * 128, (nt + 1) * 128)
                ps = mm512.tile([128, C], f32, tag="mm512", name="mm512")
                nc.tensor.matmul(ps[:], yf[0][:, s], wvT[0][:], start=True, stop=False)
                nc.tensor.matmul(ps[:], yf[1][:, s], wvT[1][:], start=False, stop=True)
                nc.vector.tensor_copy(vT[nt][:], ps[:])

            # ---- main attention loop ----
            for ib in range(N_IB):
                ibs = slice(ib * I_BLK, (ib + 1) * I_BLK)
                out_ps = [outps.tile([128, I_BLK], f32, tag="outps", name="outps") for _ in range(2)]
                sacc = sacc_pool.tile([128, I_BLK], f32, tag="sacc", name="sacc")

                for jt in range(N_JT):
                    jts = slice(jt * 128, (jt + 1) * 128)
                    eT = mm512.tile([128, I_BLK], f32, tag="mm512", name="mm512")
                    nc.tensor.matmul(eT[:], k_sb[0][:, jts], qT[0][:, ibs],
                                     start=True, stop=False)
                    nc.tensor.matmul(eT[:], k_sb[1][:, jts], qT[1][:, ibs],
                                     start=False, stop=True)
                    pT = ptile.tile([128, I_BLK], f32r, tag="pT", name="pT")
                    nc.scalar.activation(pT[:], eT[:], Exp, bias=neg_m[:], scale=1.0)
                    if jt == 0:
                        nc.vector.tensor_copy(sacc[:], pT[:])
                    else:
                        nc.vector.tensor_add(sacc[:], sacc[:], pT[:])
                    for cc in range(2):
                        ccs = slice(cc * 128, (cc + 1) * 128)
                        nc.tensor.matmul(out_ps[cc][:], vT[jt][:, ccs], pT[:],
                                         start=(jt == 0), stop=(jt == N_JT - 1))

                # softmax denominator: s[i] = sum over partitions of sacc
                s_ps = sps.tile([1, I_BLK], f32, tag="sr", name="sps")
                nc.tensor.matmul(s_ps[:], ones_col[:], sacc[:], start=True, stop=True)
                r_row = rrow_pool.tile([1, I_BLK], f32, tag="rrow", name="rrow")
                nc.vector.reciprocal(r_row[:], s_ps[:])
                r_bc = sps.tile([128, I_BLK], f32, tag="sr", name="rbc")
                nc.tensor.matmul(r_bc[:], ones_row[:], r_row[:], start=True, stop=True)
                r_bc_sb = fin.tile([128, I_BLK], f32, tag="rbcsb", name="rbcsb")
                nc.scalar.copy(r_bc_sb[:], r_bc[:])

                for cc in range(2):
                    rows = slice(cc * 128, (cc + 1) * 128)
                    tmp = fin.tile([128, I_BLK], f32, tag="tmp", name="tmp")
                    nc.vector.tensor_tensor(tmp[:], out_ps[cc][:], r_bc_sb[:], Mult)
                    final = fin.tile([128, I_BLK], f32, tag="final", name="final")
                    nc.vector.tensor_add(final[:], tmp[:],
                                         xf[cc][:, ibs].bitcast(f32))
                    nc.sync.dma_start(out=out_d[rows, ibs], in_=final[:])

    nc.compile()
    return nc


def kernel(x, y, wq, wk, wv, gamma):
    from concourse.bass_utils import run_bass_kernel_spmd

    if "nc" not in _CACHE:
        _CACHE["nc"] = _build()
    nc = _CACHE["nc"]

    x = np.asarray(x, dtype=np.float32)
    y = np.asarray(y, dtype=np.float32)
    wqT = np.ascontiguousarray(np.asarray(wq, np.float32).T)
    wkT = np.ascontiguousarray(np.asarray(wk, np.float32).T)
    wvT = np.ascontiguousarray(np.asarray(wv, np.float32).T * np.float32(gamma[0]))

    in_maps = []
    for c in range(8):
        b, h = divmod(c, 2)
        xfb = x[b].reshape(C, N)
        in_maps.append({
            "xf": np.ascontiguousarray(xfb[:, h * NQ:(h + 1) * NQ]),
            "yf": np.ascontiguousarray(y[b].reshape(C, N)),
            "wqT": wqT,
            "wkT": wkT,
            "wvT": wvT,
        })

    res = run_bass_kernel_spmd(nc, in_maps, list(range(8)))

    out = np.empty((B, C, N), dtype=np.float32)
    for c in range(8):
        b, h = divmod(c, 2)
        out[b][:, h * NQ:(h + 1) * NQ] = res.results[c]["out"]
    return out.reshape(B, C, 64, 64)


# revision 10
# speedup vs baseline: 7738.7309x; 7738.7309x over previous
"""TRN2 Bass kernel for CrossAttentionBlock.

Reference computation (per batch b):
  q = (wq @ xf)          # [Co, N] -> used transposed
  k = wk @ yf            # [Co, N]
  v = wv @ yf            # [Co, N]
  energy[i, j] = sum_o q[o, i] * k[o, j]
  att = softmax_j(energy)
  out[c, i] = gamma * sum_j v[c, j] * att[i, j] + x[c, i]

Sharding: 8 cores = 4 batches x 2 query-halves. Each core handles the
2048 query rows of one half of one batch; the full [Co, N] k/v for that
batch are computed on-core (cheap projections, duplicated per pair).

On-core dataflow (all matmuls in float32r: 1 cycle/row at free-dim>=256,
~2e-4 relative error vs 2.7e-3 for bf16):
  - qT [o, i] and k [o, j]: contraction dim o lives on partitions.
  - energy computed transposed, eT [j_tile=128, i_blk] per j-tile, so the
    softmax exp can stream PSUM->SBUF through the ACT engine with a
    global-max subtraction (M=60; softmax is shift-invariant, rows cannot
    underflow to zero for this energy scale).
  - attention-weighted V accumulates in natural [c, i] layout across the
    j-loop: lhsT = vT[j, c_chunk], rhs = pT[j, i_blk].
  - row sums: DVE accumulates pT tiles elementwise (two alternating
    accumulators to halve the serial chain); a ones-vector matmul reduces
    over partitions; reciprocal + a rank-1 ones matmul broadcasts 1/s back
    across partitions; finalize fuses *1/s and +x on DVE.
  - gamma is folded into wv on the host.
"""

import numpy as np

B = 4
C = 256
N = 4096          # H * W
NQ = N // 2       # query rows per core
I_BLK = 1024
N_IB = NQ // I_BLK   # 2 i-blocks
N_JT = N // 128      # 32 j-tiles
NEG_M = -60.0        # global softmax shift

_CACHE = {}


def _build(reps=1):
    import concourse.tile as tile
    from concourse import bacc, mybir

    f32 = mybir.dt.float32
    f32r = mybir.dt.float32r
    Exp = mybir.ActivationFunctionType.Exp
    Mult = mybir.AluOpType.mult

    nc = bacc.Bacc("TRN2", target_bir_lowering=False, debug=False)

    xf_d = nc.dram_tensor("xf", [C, NQ], f32r, kind="ExternalInput")
    yf_d = nc.dram_tensor("yf", [C, N], f32r, kind="ExternalInput")
    wqT_d = nc.dram_tensor("wqT", [C, C], f32r, kind="ExternalInput")
    wkT_d = nc.dram_tensor("wkT", [C, C], f32r, kind="ExternalInput")
    wvT_d = nc.dram_tensor("wvT", [C, C], f32r, kind="ExternalInput")
    out_d = nc.dram_tensor("out", [C, NQ], f32, kind="ExternalOutput")

    with tile.TileContext(nc) as tc:
        with (
            tc.tile_pool(name="persist", bufs=1) as persist,
            tc.tile_pool(name="ptile", bufs=4) as ptile,
            tc.tile_pool(name="sacc_pool", bufs=2) as sacc_pool,
            tc.tile_pool(name="fin", bufs=2) as fin,
            tc.tile_pool(name="rrow", bufs=2) as rrow_pool,
            tc.tile_pool(name="mmps", bufs=2, space="PSUM") as mmps,
            tc.tile_pool(name="outps", bufs=2, space="PSUM") as outps,
        ):
            # ---- load inputs (split along free dim so compute starts early) ----
            xf = [persist.tile([128, NQ], f32r, tag=f"xf{cc}", name=f"xf{cc}") for cc in range(2)]
            yf = [persist.tile([128, N], f32r, tag=f"yf{cc}", name=f"yf{cc}") for cc in range(2)]
            wqT = [persist.tile([128, C], f32r, tag=f"wq{cc}", name=f"wq{cc}") for cc in range(2)]
            wkT = [persist.tile([128, C], f32r, tag=f"wk{cc}", name=f"wk{cc}") for cc in range(2)]
            wvT = [persist.tile([128, C], f32r, tag=f"wv{cc}", name=f"wv{cc}") for cc in range(2)]
            for cc in range(2):
                rows = slice(cc * 128, (cc + 1) * 128)
                nc.sync.dma_start(out=wqT[cc][:], in_=wqT_d[rows, :])
                nc.sync.dma_start(out=wkT[cc][:], in_=wkT_d[rows, :])
                nc.sync.dma_start(out=wvT[cc][:], in_=wvT_d[rows, :])
                for h in range(2):
                    nc.sync.dma_start(out=xf[cc][:, h * 1024:(h + 1) * 1024],
                                      in_=xf_d[rows, h * 1024:(h + 1) * 1024])
                    nc.sync.dma_start(out=yf[cc][:, h * 2048:(h + 1) * 2048],
                                      in_=yf_d[rows, h * 2048:(h + 1) * 2048])

            ones_col = persist.tile([128, 1], f32, tag="ones_col", name="ones_col")
            nc.vector.memset(ones_col[:], 1.0)
            ones_row = persist.tile([1, 128], f32, tag="ones_row", name="ones_row")
            nc.vector.memset(ones_row[:], 1.0)
            neg_m = persist.tile([128, 1], f32, tag="neg_m", name="neg_m")
            nc.vector.memset(neg_m[:], NEG_M)

            qT = [persist.tile([128, NQ], f32r, tag=f"qT{oc}", name=f"qT{oc}") for oc in range(2)]
            k_sb = [persist.tile([128, N], f32r, tag=f"k{oc}", name=f"k{oc}") for oc in range(2)]
            # vT_all[p, nt, c] = v[nt*128 + p, c]
            vT_all = persist.tile([128, N_JT, C], f32r, tag="vT", name="vT_all")

            for _rep in range(reps):
                # ---- projections ----
                for oc in range(2):
                    ocs = slice(oc * 128, (oc + 1) * 128)
                    for it in range(2):
                        ps = mmps.tile([128, 1024], f32, tag="mmps", name="q_ps")
                        for hh in range(2):
                            s = slice(it * 1024 + hh * 512, it * 1024 + (hh + 1) * 512)
                            d = slice(hh * 512, (hh + 1) * 512)
                            nc.tensor.matmul(ps[:, d], wqT[0][:, ocs], xf[0][:, s],
                                             start=True, stop=False)
                            nc.tensor.matmul(ps[:, d], wqT[1][:, ocs], xf[1][:, s],
                                             start=False, stop=True)
                        nc.scalar.copy(qT[oc][:, it * 1024:(it + 1) * 1024], ps[:])
                    for jc in range(4):
                        ps = mmps.tile([128, 1024], f32, tag="mmps", name="k_ps")
                        for hh in range(2):
                            s = slice(jc * 1024 + hh * 512, jc * 1024 + (hh + 1) * 512)
                            d = slice(hh * 512, (hh + 1) * 512)
                            nc.tensor.matmul(ps[:, d], wkT[0][:, ocs], yf[0][:, s],
                                             start=True, stop=False)
                            nc.tensor.matmul(ps[:, d], wkT[1][:, ocs], yf[1][:, s],
                                             start=False, stop=True)
                        nc.vector.tensor_copy(k_sb[oc][:, jc * 1024:(jc + 1) * 1024], ps[:])
                for ng in range(N_JT // 4):
                    ps = mmps.tile([128, 1024], f32, tag="mmps", name="v_ps")
                    for sub in range(4):
                        nt = ng * 4 + sub
                        s = slice(nt * 128, (nt + 1) * 128)
                        d = slice(sub * C, (sub + 1) * C)
                        nc.tensor.matmul(ps[:, d], yf[0][:, s], wvT[0][:],
                                         start=True, stop=False)
                        nc.tensor.matmul(ps[:, d], yf[1][:, s], wvT[1][:],
                                         start=False, stop=True)
                    nc.vector.tensor_copy(
                        vT_all[:, ng * 4:(ng + 1) * 4, :], ps[:])

                # ---- main attention loop ----
                for ib in range(N_IB):
                    ibs = slice(ib * I_BLK, (ib + 1) * I_BLK)
                    out_ps = [outps.tile([128, I_BLK], f32, tag="outps", name="outps")
                              for _ in range(2)]
                    sacc0 = sacc_pool.tile([128, I_BLK], f32, tag="sacc0", name="sacc0")
                    sacc1 = sacc_pool.tile([128, I_BLK], f32, tag="sacc1", name="sacc1")

                    for jt in range(N_JT):
                        jts = slice(jt * 128, (jt + 1) * 128)
                        eT = mmps.tile([128, I_BLK], f32, tag="mmps", name="eT")
                        for hh in range(2):
                            d = slice(hh * 512, (hh + 1) * 512)
                            s = slice(ib * I_BLK + hh * 512, ib * I_BLK + (hh + 1) * 512)
                            nc.tensor.matmul(eT[:, d], k_sb[0][:, jts], qT[0][:, s],
                                             start=True, stop=False)
                            nc.tensor.matmul(eT[:, d], k_sb[1][:, jts], qT[1][:, s],
                                             start=False, stop=True)
                        pT = ptile.tile([128, I_BLK], f32r, tag="pT", name="pT")
                        nc.scalar.activation(pT[:], eT[:], Exp, bias=neg_m[:], scale=1.0)
                        sacc = sacc0 if jt % 2 == 0 else sacc1
                        if jt < 2:
                            nc.vector.tensor_copy(sacc[:], pT[:].bitcast(f32))
                        else:
                            nc.vector.tensor_add(sacc[:], sacc[:], pT[:].bitcast(f32))
                        for cc in range(2):
                            ccs = slice(cc * 128, (cc + 1) * 128)
                            for hh in range(2):
                                d = slice(hh * 512, (hh + 1) * 512)
                                nc.tensor.matmul(out_ps[cc][:, d],
                                                 vT_all[:, jt, ccs], pT[:, d],
                                                 start=(jt == 0), stop=(jt == N_JT - 1))

                    # softmax denominator: s[i] = sum over partitions of sacc
                    sacc_m = sacc_pool.tile([128, I_BLK], f32, tag="saccm", name="saccm")
                    nc.vector.tensor_add(sacc_m[:], sacc0[:], sacc1[:])
                    s_ps = mmps.tile([1, I_BLK], f32, tag="mmps", name="s_ps")
                    for hh in range(2):
                        d = slice(hh * 512, (hh + 1) * 512)
                        nc.tensor.matmul(s_ps[:, d], ones_col[:], sacc_m[:, d],
                                         start=True, stop=True)
                    r_row = rrow_pool.tile([1, I_BLK], f32, tag="rrow", name="rrow")
                    nc.vector.reciprocal(r_row[:], s_ps[:])
                    r_bc = mmps.tile([128, I_BLK], f32, tag="mmps", name="rbc")
                    for hh in range(2):
                        d = slice(hh * 512, (hh + 1) * 512)
                        nc.tensor.matmul(r_bc[:, d], ones_row[:], r_row[:, d],
                                         start=True, stop=True)
                    r_bc_sb = fin.tile([128, I_BLK], f32, tag="rbcsb", name="rbcsb")
                    nc.scalar.copy(r_bc_sb[:], r_bc[:])

                    for cc in range(2):
                        rows = slice(cc * 128, (cc + 1) * 128)
                        final = fin.tile([128, I_BLK], f32, tag="final", name="final")
                        nc.vector.tensor_tensor(final[:], out_ps[cc][:], r_bc_sb[:], Mult)
                        nc.vector.tensor_add(final[:], final[:],
                                             xf[cc][:, ibs].bitcast(f32))
                        nc.sync.dma_start(out=out_d[rows, ibs], in_=final[:])

    nc.compile()
    return nc


def kernel(x, y, wq, wk, wv, gamma):
    from concourse.bass_utils import run_bass_kernel_spmd

    if "nc" not in _CACHE:
        _CACHE["nc"] = _build()
    nc = _CACHE["nc"]

    x = np.asarray(x, dtype=np.float32)
    y = np.asarray(y, dtype=np.float32)
    wqT = np.ascontiguousarray(np.asarray(wq, np.float32).T)
    wkT = np.ascontiguousarray(np.asarray(wk, np.float32).T)
    wvT = np.ascontiguousarray(np.asarray(wv, np.float32).T * np.float32(gamma[0]))

    in_maps = []
    for c in range(8):
        b, h = divmod(c, 2)
        xfb = x[b].reshape(C, N)
        in_maps.append({
            "xf": np.ascontiguousarray(xfb[:, h * NQ:(h + 1) * NQ]),
            "yf": np.ascontiguousarray(y[b].reshape(C, N)),
            "wqT": wqT,
            "wkT": wkT,
            "wvT": wvT,
        })

    res = run_bass_kernel_spmd(nc, in_maps, list(range(8)))

    out = np.empty((B, C, N), dtype=np.float32)
    for c in range(8):
        b, h = divmod(c, 2)
        out[b][:, h * NQ:(h + 1) * NQ] = res.results[c]["out"]
    return out.reshape(B, C, 64, 64)
